# revision 52
# baseline (speedup 1.0000x reference)
"""8-core Trainium2 Bass kernel for causal multi-head attention.

Problem: B=4, S=2048, E=1024, H=16 heads, D=64.
  y = softmax(causal(Q K^T / sqrt(D))) V, with Q/K/V/O linear projections.

Sharding (hardcoded): hybrid batch x head split over 8 cores.
  core c -> batch b = c % 4, head-group hg = c // 4 (8 heads each).
Each core computes its batch's attention output for its 8 heads plus the
partial output projection y_partial = attn_local @ Wo[:, cslice].T.
Host sums the two partials per batch (Megatron-style TP reduce done on host).

Device layouts (host pre-transposes/casts to bf16):
  xT   [E, S]   = x[b].T
  wqT  [E, CL]  = Wq[cslice, :].T     (CL = 512 local channels)
  woT  [CL, E]  = Wo[:, cslice].T

Attention is computed fully transposed: scores^T [k, q] strips via
matmul(lhsT=K^T k-tile, rhs=Q^T), exp on ScalarE in 1024-wide chunks
(scale=1/8 folded in; no max-subtraction: |scores| <~ 4 at this weight
scale), causality by only computing q >= k-tile strips + one 128x128
triangular mask multiply per strip.  PV uses V augmented with a ones
column so the softmax denominator accumulates in PSUM row 64 for free.
The attn output lands directly in [c, s] layout = lhsT of the Wo matmul.

The kt-outer loop accumulates all 4 q-chunk PSUM tiles at once; the Q/K
projections of the NEXT head-pair are interleaved into the current pair's
attention stream to keep TensorE busy while ScalarE (exp) is the local
bottleneck.

Perf refinements over the original version (317us -> 289us):
  * V/Wo bias adds folded into the PSUM-drain TensorTensor on DVE against
    broadcast bias planes (drops the K=1 bias matmuls from the PE stream).
  * y partials emitted in bf16 (halves output DMA bytes); host upcasts and
    sums in f32.
  * y writeback DMAs ride the Pool SWDGE queue mid-attention and the
    Activation hwdge queue during the final drain, so they never head-block
    the SP queue carrying the normalize partition-bounce DMAs (in-order DMA
    queues otherwise cascade: y waits yt-TT, blocks attn bounce, stalls Wo).
  * deeper yt/exp/small pools decouple PSUM-drain WARs from DMA latency.
  * consolidated input loads (one large DMA per tensor, x seq-chunk-major,
    bias planes slotted by first-use deadline).
  * Wo pops delayed 2 extra strips so their Ldweights never reach the PE
    queue head before the attn normalize chain has landed.
  * (289us -> 285us) startup shortened: only V(0..7)+QK0 run before
    attention; V(8..15) interleaves into pair-0 h0's strip stream and the
    whole pair-1 Q/K projection into pair-0 h1, through explicitly
    scheduled po slots (po0 free from kt>=5, po1 from kt>=8 -- the old
    toggle could hit a still-live accumulator and head-block PE).
  * (242us -> 240us) the three narrowest score strips (kt 13-15) share
    one PSUM tile at bank-aligned offsets and a single exp instruction
    (the ~185ns per-instruction Activation init is the mid-run bottleneck
    now that the windows are exp-bound; the offset-gap columns are exp'd
    garbage that PV never reads).
  * (275us -> 242us) the Q/K projections run as fp8e4 DoubleRow matmuls
    (0.5 cycles/col, two 128-deep contraction tiles per instruction): the
    host ships fp8 copies of x and Wq/Wk; DoubleRow's [K, 2, N] operand
    layout maps directly onto the existing [P, EO, seq] SBUF tiling, so
    only the projection matmuls change -- qT/kT stay bf16 downstream.
    Q/K quantization only perturbs softmax weights (V/Wo stay bf16);
    rel err 0.0043 -> 0.0138, still 1.4x under the 2e-2 gate.
  * (285us -> 275us) a leading [*, 0:128] x mini-DMA lets V(st=0) start
    ~5us earlier (disjoint region, so no WAR against the follow-up load);
    pair-3's Wo chunks run through the po slots (free there -- pair 3 has
    no projection interleave) instead of contending with the scores
    double-buffer, both for mid-attention pops and the tail drain.
"""

import functools

import ml_dtypes
import numpy as np

import concourse.bacc as bacc
import concourse.mybir as mybir
import concourse.tile as tile
from concourse.bass_utils import run_bass_kernel_spmd
from concourse.masks import make_upper_triangular

B, S, E, H, D = 4, 2048, 1024, 16, 64
NCORES = 8
HL = H // 2  # local heads per core
CL = HL * D  # 512 local channels
P = 128
QCW = 512  # q-chunk width (one PSUM bank of fp32)
F32 = mybir.dt.float32
BF16 = mybir.dt.bfloat16
F8 = mybir.dt.float8e4
F8NP = mybir.dt.np(mybir.dt.float8e4)
BF = ml_dtypes.bfloat16
EO = E // P  # 8 contraction tiles for projections
CT = CL // P  # 4 c-tiles (head pairs)


def build_mha_core(seq: int = S):
    assert seq % QCW == 0
    NQC = seq // QCW
    NST = seq // P

    nc = bacc.Bacc(None, target_bir_lowering=False)
    xT_d = nc.dram_tensor("xT", [E, seq], BF16, kind="ExternalInput")
    wqT_d = nc.dram_tensor("wqT", [E, CL], F8, kind="ExternalInput")
    wkT_d = nc.dram_tensor("wkT", [E, CL], F8, kind="ExternalInput")
    xT8_d = nc.dram_tensor("xT8", [E, seq], F8, kind="ExternalInput")
    wvT_d = nc.dram_tensor("wvT", [E, CL], BF16, kind="ExternalInput")
    woT_d = nc.dram_tensor("woT", [CL, E], BF16, kind="ExternalInput")
    bq_d = nc.dram_tensor("bq", [CL], F32, kind="ExternalInput")
    bk_d = nc.dram_tensor("bk", [CL], F32, kind="ExternalInput")
    bv_d = nc.dram_tensor("bv", [CL], F32, kind="ExternalInput")
    bo_d = nc.dram_tensor("bo", [E], F32, kind="ExternalInput")
    y_d = nc.dram_tensor("y", [seq, E], BF16, kind="ExternalOutput")

    with tile.TileContext(nc) as tc:
        with (
            tc.tile_pool(name="singles", bufs=1) as singles,
            tc.tile_pool(name="exp_pool", bufs=6) as exp_pool,
            tc.tile_pool(name="yt_pool", bufs=6) as yt_pool,
            tc.tile_pool(name="small1", bufs=4) as small1,
            tc.tile_pool(name="dram", bufs=1, space="DRAM") as dram_pool,
            tc.tile_pool(name="psum_main", bufs=2, space="PSUM") as psum_main,
            tc.tile_pool(name="psum_acc", bufs=1, space="PSUM") as psum_acc,
        ):
            # ---------- constants ----------
            # broadcast bias planes: bias adds ride the PSUM-drain TensorTensor
            # on DVE instead of K=1 matmuls on the (bottleneck) PE
            bv_bc = singles.tile([P, CL], F32)
            bo_bc = singles.tile([P, E], F32)
            mask_sb = singles.tile([P, P], BF16)  # 1 where q >= k (within block)
            make_upper_triangular(nc, mask_sb[:], val=1.0, diag=True)
            bqk_sb = singles.tile([P, 2, CT], F32)

            # ---------- SBUF residents ----------
            xT_sb = singles.tile([P, EO, seq], BF16)
            xT_ap = xT_d[:].rearrange("(eo p) s -> eo p s", p=P)
            wq_sb = singles.tile([P, EO, CL], F8)
            wk_sb = singles.tile([P, EO, CL], F8)
            x8_sb = singles.tile([P, EO, seq], F8)
            wv_sb = singles.tile([P, EO, CL], BF16)
            wo_sb = singles.tile([P, CT, E], BF16)
            x_src = xT_ap.rearrange("eo p s -> p eo s")
            nc.sync.dma_start(wv_sb[:], wvT_d[:].rearrange("(eo p) c -> p eo c", p=P))
            # leading mini-chunk so V(st=0) isn't gated on the full first
            # s-chunk transfer (the regions are disjoint: no WAR between the
            # first V reads and the follow-up load)
            nc.sync.dma_start(xT_sb[:, :, 0:P], x_src[:, :, 0:P])
            nc.sync.dma_start(xT_sb[:, :, P:512], x_src[:, :, P:512])
            nc.sync.dma_start(bv_bc[:], bv_d[None, :].to_broadcast((P, CL)))
            nc.sync.dma_start(xT_sb[:, :, 512:1024], x_src[:, :, 512:1024])
            x8_src = xT8_d[:].rearrange("(eo p) s -> p eo s", p=P)
            nc.sync.dma_start(wq_sb[:], wqT_d[:].rearrange("(eo p) c -> p eo c", p=P))
            nc.sync.dma_start(x8_sb[:, :, 0:1024], x8_src[:, :, 0:1024])
            nc.sync.dma_start(wk_sb[:], wkT_d[:].rearrange("(eo p) c -> p eo c", p=P))
            nc.sync.dma_start(xT_sb[:, :, 1024:1536], x_src[:, :, 1024:1536])
            nc.sync.dma_start(xT_sb[:, :, 1536:seq], x_src[:, :, 1536:seq])
            nc.sync.dma_start(x8_sb[:, :, 1024:seq], x8_src[:, :, 1024:seq])
            nc.sync.dma_start(bqk_sb[:, 0], bq_d[:].rearrange("(ct p) -> p ct", p=P))
            nc.sync.dma_start(bqk_sb[:, 1], bk_d[:].rearrange("(ct p) -> p ct", p=P))
            nc.sync.dma_start(wo_sb[:], woT_d[:].rearrange("(ct p) e -> p ct e", p=P))
            nc.sync.dma_start(bo_bc[:], bo_d[None, :].to_broadcast((P, E)))

            # per-pair Q^T/K^T tiles (separate tiles -> no false WAR deps
            # when the next pair's projection interleaves with attention)
            qT_sb = [singles.tile([P, seq], BF16, name=f"qT{i}") for i in range(CT)]
            kT_sb = [singles.tile([P, seq], BF16, name=f"kT{i}") for i in range(CT)]
            v_sb = singles.tile([P, NST, HL, D + 1], BF16)
            attn_sb = singles.tile([P, CT, seq], BF16)
            rec_dram = dram_pool.tile([HL, seq], F32)

            nc.vector.memset(v_sb[:, :, :, D : D + 1], 1.0)

            # ---------- V projection:  v[s, c] (+ ones column) ----------
            def emit_v_step(st):
                ps = psum_main.tile([P, 2 * QCW], F32, tag="mm", name="v_ps")
                for eo in range(EO):
                    nc.tensor.matmul(
                        ps[:, :QCW],
                        xT_sb[:, eo, st * P : (st + 1) * P],
                        wv_sb[:, eo, :],
                        start=(eo == 0),
                        stop=(eo == EO - 1),
                    )
                nc.vector.tensor_add(
                    v_sb[:, st, :, 0:D],
                    ps[:, :QCW].rearrange("p (h d) -> p h d", d=D),
                    bv_bc[:].rearrange("p (h d) -> p h d", d=D),
                )

            # V(0..7) upfront; V(8..15) interleaves into pair-0 h0's
            # attention stream (slots in the schedule below)
            for st in range(8):
                emit_v_step(st)

            # ---------- Q^T/K^T projection steps (generator per pair) ----------
            def qk_steps(pair):
                """Yield 2*NQC emission steps; each computes one [128, QCW]
                chunk of Q^T or K^T for this pair (= c-tile)."""
                for which, w_sb, outT in ((0, wq_sb, qT_sb), (1, wk_sb, kT_sb)):
                    for sc in range(NQC):
                        yield which, w_sb, outT, sc

            def emit_qk_step(step, pair, slot):
                which, w_sb, outT, sc = step
                ps = psum_acc.tile([P, QCW], F32, tag=f"po{slot}", name="qk_ps")
                for p8 in range(EO // 2):
                    nc.tensor.matmul(
                        ps[:],
                        w_sb[:, 2 * p8 : 2 * p8 + 2, pair * P : (pair + 1) * P],
                        x8_sb[:, 2 * p8 : 2 * p8 + 2, sc * QCW : (sc + 1) * QCW],
                        start=(p8 == 0),
                        stop=(p8 == EO // 2 - 1),
                        perf_mode=mybir.MatmulPerfMode.DoubleRow,
                    )
                nc.vector.tensor_scalar_add(
                    outT[pair][:, sc * QCW : (sc + 1) * QCW],
                    ps[:],
                    bqk_sb[:, which, pair : pair + 1],
                )

            def emit_wo(st, dma_eng=None, po_slots=None):
                """Partial output projection for one 128-row s-tile."""
                for ec in range(E // QCW):
                    if po_slots is None:
                        ps = psum_main.tile([P, 2 * QCW], F32, tag="mm", name="wo_ps")
                    else:
                        ps = psum_acc.tile(
                            [P, QCW], F32, tag=f"po{next(po_slots)}", name="wo_ps"
                        )
                    for ct in range(CT):
                        nc.tensor.matmul(
                            ps[:, :QCW],
                            attn_sb[:, ct, st * P : (st + 1) * P],
                            wo_sb[:, ct, ec * QCW : (ec + 1) * QCW],
                            start=(ct == 0),
                            stop=(ct == CT - 1),
                        )
                    yt = yt_pool.tile([P, QCW], BF16, tag="yt")
                    nc.vector.tensor_add(
                        yt[:], ps[:, :QCW], bo_bc[:, ec * QCW : (ec + 1) * QCW]
                    )
                    # mid-attention pops ride SP; the tail drain rides the
                    # Activation hwdge queue (idle once the last exp is done).
                    # Pool must stay clear: it carries the normalize broadcast.
                    (dma_eng or nc.gpsimd).dma_start(
                        y_d[st * P : (st + 1) * P, ec * QCW : (ec + 1) * QCW],
                        yt[:],
                    )

            def emit_v_ilv(st, slot):
                """V s-tile through a freed attention po slot."""
                ps = psum_acc.tile([P, QCW], F32, tag=f"po{slot}", name="v_ps_i")
                for eo in range(EO):
                    nc.tensor.matmul(
                        ps[:],
                        xT_sb[:, eo, st * P : (st + 1) * P],
                        wv_sb[:, eo, :],
                        start=(eo == 0),
                        stop=(eo == EO - 1),
                    )
                nc.vector.tensor_add(
                    v_sb[:, st, :, 0:D],
                    ps[:].rearrange("p (h d) -> p h d", d=D),
                    bv_bc[:].rearrange("p (h d) -> p h d", d=D),
                )

            # interleave schedules per head-loop: (kt-1 trigger, po slot, job).
            # Slot timing: po0 is free from kt>=5 (qc0 stops at kt=3, its
            # normalize drains during kt=4); po1 only from kt>=8.  The dense
            # pair-0 schedules carry the deferred V tiles / pair-1 projection.
            DENSE = [(5, 0), (7, 0), (8, 1), (9, 0), (10, 1), (11, 0), (12, 1), (13, 0)]
            SPARSE = [(5, 0), (8, 1), (11, 0), (14, 1)]

            def make_ilv(pair, hh):
                if pair == 0 and hh == 0:
                    return [(t, sl, ("v", st))
                            for (t, sl), st in zip(DENSE, range(8, NST))]
                if pair == 0 and hh == 1:
                    return [(t, sl, ("qk", step))
                            for (t, sl), step in zip(DENSE, qk_steps(1))]
                if pair + 1 < CT:
                    steps = list(qk_steps(pair + 1))
                    half = steps[:4] if hh == 0 else steps[4:]
                    return [(t, sl, ("qk", step))
                            for (t, sl), step in zip(SPARSE, half)]
                return []

            # pair 0 projected up front (po slots 0-3 round-robin)
            for i, step in enumerate(qk_steps(0)):
                emit_qk_step(step, 0, i % 4)

            # ---------- attention (kt-outer strips), work interleaved ----------
            for pair in range(CT):
                for hh in range(2):
                    ilv = make_ilv(pair, hh)
                    h = 2 * pair + hh
                    hp = hh * 64
                    po = [
                        psum_acc.tile(
                            [D + 1, QCW], F32, tag=f"po{qc}", name=f"po{qc}"
                        )
                        for qc in range(NQC)
                    ]
                    def emit_strip(kt):
                        """scores^T strip [k=128, q in [kt*P, seq)] -> exp -> et."""
                        kq0 = kt * P
                        W = seq - kq0
                        et = exp_pool.tile([P, seq], BF16, tag="exp", name="et")
                        pos = 0
                        while pos < W:
                            cw = min(2 * QCW, W - pos)
                            ps = psum_main.tile(
                                [P, 2 * QCW], F32, tag="mm", name="sc_ps"
                            )
                            for j0 in range(0, cw, QCW):
                                jw = min(QCW, cw - j0)
                                nc.tensor.matmul(
                                    ps[:, j0 : j0 + jw],
                                    kT_sb[pair][hp : hp + D, kq0 : kq0 + P],
                                    qT_sb[pair][
                                        hp : hp + D,
                                        kq0 + pos + j0 : kq0 + pos + j0 + jw,
                                    ],
                                )
                            nc.scalar.activation(
                                et[:, pos : pos + cw],
                                ps[:, :cw],
                                mybir.ActivationFunctionType.Exp,
                                scale=float(D) ** -0.5,
                            )
                            pos += cw
                        # causal mask on the diagonal block (strip-local 0:128)
                        nc.vector.tensor_mul(et[:, 0:P], et[:, 0:P], mask_sb[:])
                        return et

                    def emit_pv(kt, et):
                        """PV updates into every q-chunk this k-tile touches."""
                        kq0 = kt * P
                        for qc in range(kt // (QCW // P), NQC):
                            off = max(0, kq0 - qc * QCW)
                            s0 = qc * QCW + off - kq0
                            last = kt == qc * (QCW // P) + (QCW // P) - 1
                            nc.tensor.matmul(
                                po[qc][:, off:],
                                v_sb[:, kt, h, :],
                                et[:, s0 : s0 + QCW - off],
                                start=(kt == 0),
                                stop=last,
                            )
                            if last:
                                _normalize_chunk(
                                    nc, h, hp, pair, qc, po[qc],
                                    attn_sb, rec_dram, small1,
                                )
                                if h == HL - 1:
                                    # last head: attn for these s-tiles is now
                                    # final across all pairs; queue Wo and pop
                                    # later so its normalize->DMA chain clears
                                    # before the Wo matmuls enter the PE FIFO
                                    wo_pending.extend(
                                        range(
                                            qc * (QCW // P),
                                            (qc + 1) * (QCW // P),
                                        )
                                    )

                    def emit_tail_strips():
                        """kt 13/14/15 merged: one scores tile, one exp."""
                        ps = psum_main.tile([P, 2 * QCW], F32, tag="mm", name="sc_ps")
                        et = exp_pool.tile([P, seq], BF16, tag="exp", name="et")
                        parts = [(13, 0, 384), (14, 512, 256), (15, 768, 128)]
                        for kt, off, W in parts:
                            kq0 = kt * P
                            nc.tensor.matmul(
                                ps[:, off : off + W],
                                kT_sb[pair][hp : hp + D, kq0 : kq0 + P],
                                qT_sb[pair][hp : hp + D, kq0 : kq0 + W],
                            )
                        nc.scalar.activation(
                            et[:, 0:896],
                            ps[:, 0:896],
                            mybir.ActivationFunctionType.Exp,
                            scale=float(D) ** -0.5,
                        )
                        out = {}
                        for kt, off, W in parts:
                            nc.vector.tensor_mul(
                                et[:, off : off + P], et[:, off : off + P], mask_sb[:]
                            )
                            out[kt] = et[:, off : off + W]
                        return out

                    # software pipeline: scores(kt+1) issued before PV(kt) so
                    # the PE FIFO never parks on exp(kt) with scores runnable
                    wo_pending = []
                    import itertools
                    pop_slots = itertools.cycle(range(2)) if h == HL - 1 else None
                    prev = None
                    merged = {}
                    for kt in range(NST + 1):
                        if kt == 13:
                            merged = emit_tail_strips()
                        if kt < 13:
                            cur = emit_strip(kt)
                        elif kt < NST:
                            cur = merged[kt]
                        else:
                            cur = None
                        if prev is not None:
                            emit_pv(kt - 1, prev)
                            # interleaved fill work through freed po slots
                            while ilv and ilv[0][0] == kt - 1:
                                _, slot, job = ilv.pop(0)
                                if job[0] == "v":
                                    emit_v_ilv(job[1], slot)
                                elif job[0] == "qk0":
                                    emit_qk_step(job[1], 0, slot)
                                else:
                                    emit_qk_step(job[1], pair + 1, slot)
                            # pop one queued Wo s-tile, >= 2 kts after its
                            # normalize was issued
                            if wo_pending and kt - 1 >= (wo_pending[0] // 4) * 4 + 7:
                                emit_wo(wo_pending.pop(0), po_slots=pop_slots)
                        prev = cur
                    drain_slots = None
                    if h == HL - 1:
                        import itertools
                        drain_slots = itertools.cycle(range(4))
                    for st in wo_pending:
                        emit_wo(
                            st,
                            dma_eng=nc.scalar if h == HL - 1 else None,
                            po_slots=drain_slots,
                        )
                    assert not ilv, "interleave schedule not drained" 

    nc.compile()
    return nc


def _normalize_chunk(nc, h, hp, pair, qc, po, attn_sb, rec_dram, small1):
    """attn[c, q] = po[d, q] * (1 / sums[q]); sums live in po row D.

    The PSUM tile is drained immediately (reciprocal + raw copy) so its bank
    frees fast; the 1/sums broadcast (DRAM round trip — DVE cannot shift
    partitions, DMA cannot read PSUM) then multiplies attn_sb in place.
    """
    q0 = qc * QCW
    attn_slice = attn_sb[hp : hp + D, pair, q0 : q0 + QCW]
    srow = small1.tile([P, QCW], F32, tag="srow")
    nc.vector.reciprocal(srow[D : D + 1, :], po[D : D + 1, :])
    # raw (unnormalized) copy drains the PSUM tile immediately
    if hp == 0:
        nc.vector.tensor_copy(attn_slice, po[0:D, :])
    else:
        # DVE cannot shift partitions; bounce via DMA
        tmp = small1.tile([D, QCW], BF16, tag="tmp")
        nc.vector.tensor_copy(tmp[:], po[0:D, :])
        nc.sync.dma_start(attn_slice, tmp[:])
    # 1/sums partition-broadcast via DRAM round trip (DVE cannot shift
    # partitions, DMA cannot read PSUM), then normalize attn in place
    nc.sync.dma_start(rec_dram[h, q0 : q0 + QCW], srow[D : D + 1, :])
    rb = small1.tile([P, QCW], F32, tag="rb")
    nc.sync.dma_start(
        rb[hp : hp + D, :],
        rec_dram[h, q0 : q0 + QCW][None, :].to_broadcast((D, QCW)),
    )
    nc.vector.tensor_mul(attn_slice, attn_slice, rb[hp : hp + D, :])


@functools.lru_cache(maxsize=2)
def _get_nc(seq: int):
    return build_mha_core(seq)


def make_in_maps(x, Wq, bq, Wk, bk, Wv, bv, Wo, bo, seq: int = S):
    """Shard + pre-layout the full inputs for the 8 cores."""

    def bf(a):
        return np.ascontiguousarray(a.astype(BF))

    in_maps = []
    for c in range(NCORES):
        b, hg = c % 4, c // 4
        cs = slice(hg * CL, (hg + 1) * CL)
        in_maps.append(
            {
                "xT": bf(x[b][:seq].T),
                "xT8": np.ascontiguousarray(x[b][:seq].T.astype(F8NP)),
                "wqT": np.ascontiguousarray(Wq[cs, :].T.astype(F8NP)),
                "wkT": np.ascontiguousarray(Wk[cs, :].T.astype(F8NP)),
                "wvT": bf(Wv[cs, :].T),
                "woT": bf(Wo[:, cs].T),
                "bq": np.ascontiguousarray(bq[cs], dtype=np.float32),
                "bk": np.ascontiguousarray(bk[cs], dtype=np.float32),
                "bv": np.ascontiguousarray(bv[cs], dtype=np.float32),
                "bo": np.ascontiguousarray(bo if hg == 0 else np.zeros_like(bo), dtype=np.float32),
            }
        )
    return in_maps


def kernel(x, Wq, bq, Wk, bk, Wv, bv, Wo, bo, _trace: bool = False):
    x = np.asarray(x, np.float32)
    args = [np.asarray(a, np.float32) for a in (Wq, bq, Wk, bk, Wv, bv, Wo, bo)]
    nc = _get_nc(S)
    in_maps = make_in_maps(x, *args)
    try:
        res = run_bass_kernel_spmd(
            nc, in_maps, core_ids=list(range(NCORES)), trace=_trace
        )
    except ModuleNotFoundError:
        # NTFF profiling hook unavailable in this axon client; run untraced
        res = run_bass_kernel_spmd(nc, in_maps, core_ids=list(range(NCORES)))
    outs = res.results
    y = np.empty((B, S, E), np.float32)
    for b in range(B):
        y[b] = outs[b]["y"].astype(np.float32) + outs[b + 4]["y"].astype(np.float32)
    kernel.last_exec_time_ns = res.exec_time_ns
    kernel.last_results = res
    return y

